# revision 61
# baseline (speedup 1.0000x reference)
"""Bilateral filter 3x3 (sigma_space = sigma_color = 0.8) on 8 TRN2 NeuronCores.

Sharding: pure data parallelism - one batch image [3, 512, 512] per core.

Math (per core), with the color-normalization cancelled:
  out = c + A/den
    den(x) = ws0 + sum_{k in HP} [G_k(x) + G_k(x-k)]
    A(x)   =       sum_{k in HP} [H_k(x) - H_k(x-k)]
  where HP = {E=(0,1), S=(1,0), SE=(1,1), SW=(1,-1)},
    D_k = p~(x+k) - p(x),  G_k = ws_k * exp(-D_k^2 / (2 s^2)),  H_k = D_k * G_k.

Perf design (~107.9us f32 baseline -> ~80us profiled here):
  - fp16 on-chip: host converts + transposes input to [H,C,W] fp16; output is
    fp16, upcast on the host.  DVE tensor_tensor runs in 2x_1P packed mode,
    HBM traffic halves, PE streams fp16 rhs at 1 col/cycle.
  - G in ONE ACT pass per offset via Derivative_Erf(D*s) =
    (2/sqrt(pi))*exp(-D^2/(2 sigma^2)); kappa and the per-offset spatial
    weights fold into the PE band matrices of BOTH chains (no Square pass,
    no exp bias, no extra muls).  The erf_derivative table is preloaded
    during the DMA ramp by a throwaway activation.
  - rows are loaded at both column parities (A tiles image@col2, B tiles
    image@col3, derived on DVE with alignment-free 2x_2P copies) so all four
    subs are 4-byte aligned and hit 2x.  GPSIMD handles only tiny pad ops:
    bulk GPSIMD tensor ops run ~3.5ns/elem and their SBUF traffic slows
    concurrent DVE ops ~4x.
  - shifted terms accumulate on the TensorEngine with shift-band matmuls
    into PSUM (20 passes per channel-tile); row seams across 128-row tiles
    use selector bands against the previous tile's G/H; image boundaries use
    reflect-mirror identities applied in the D domain (D odd, G even).
  - evacuation: A copied PSUM->SBUF on ACT, then one fused custom DVE op
    t8 = reciprocal1(den + ws0) * A (bitwise-NOT exponent-flip seed + one
    Newton step, 7 ALU stages, ~0.4% max err), then one packed fp16 add
    y = t8 + center.  No separate reciprocal pass, no ws0*ones matmul.
  - only 2 HBM loads per tile + 1 store (split per-channel on the last tile
    so the final, smallest store launches earliest), all HWDGE DMAs kept to
    ~16 total: more DMAs overflow the 8 round-robin HWDGE completion-sem
    lanes and serialize the whole pipeline (observed 13-33us trigger
    stalls; 20+ DMAs cost tens of us).
  - dependency-ordered engine FIFOs: the S offset (no B-copy, no pad mirror)
    leads every stage so ACT starts early; the a_sb PSUM evac sits AFTER
    DerivErf in the scalar FIFO; the custom-op evac sits AFTER the H muls in
    the vector FIFO; den chains run before A chains on PE so single-buffered
    A banks are released by the evac just in time.
  - 22 dummy matmuls during the ramp keep the PE HAM clock-gate warm
    (2.4 GHz); all 240 production matmuls then issue at ~220 ns.

Layout: partition = image rows (4 tiles x 128 rows), free = (channel, width).
WP=522 cols; A tiles: image at cols 2..513; B tiles: image at cols 3..514.
"""
import math
import numpy as np
from contextlib import ExitStack

import concourse.bacc as bacc
import concourse.tile as tile
from concourse import mybir
from concourse.bass_utils import run_bass_kernel_spmd

F32 = mybir.dt.float32
F16 = mybir.dt.float16
MM_DT = F16
MM_NP = np.float16
AF = mybir.ActivationFunctionType
ALU = mybir.AluOpType

C, H, W = 3, 512, 512
P = 128                      # partitions per row-tile
NT = H // P                  # 4 row-tiles
WP = 522                     # col-padded width
IM0 = 2                      # first image column, A-parity tiles
IMB = 3                      # first image column, B-parity tiles
J0 = IM0 - 1
J1 = IM0
J2 = IM0 + 1

SIG = 0.8
TWO_SIG2 = 2.0 * SIG * SIG   # 1.28
SCALE_SQ = 1.0 / math.sqrt(TWO_SIG2)
KAPPA = 2.0 / math.sqrt(math.pi)     # DerivErf(u) = KAPPA * exp(-u^2)
_w1 = math.exp(-1.0 / TWO_SIG2)
_norm = (1.0 + 2.0 * _w1) ** 2
WS0 = 1.0 / _norm            # center weight
WS_E = _w1 / _norm           # edge
WS_K = _w1 * _w1 / _norm     # corner
WE = WS_E / KAPPA            # band / STT scale for edge offsets
WK = WS_K / KAPPA            # band / STT scale for corner offsets

BAND_NAMES = ["b_iE", "b_isE", "b_iK", "b_sK", "b_seamD", "b_seamA",
              "b_isE0", "b_iK0",
              "b_aiE", "b_aniE", "b_ainsE", "b_aiK", "b_ansK",
              "b_ainsE0"]


def _bands_np():
    I = np.eye(P, dtype=np.float32)
    S = np.zeros((P, P), np.float32)
    for m in range(1, P):
        S[m - 1, m] = 1.0          # lhsT[p, m]: out row m <- in row m-1
    sel = np.zeros((P, P), np.float32)
    sel[P - 1, 0] = 1.0            # out row 0 <- in row 127 (prev tile)
    sel0 = np.zeros((P, P), np.float32)
    sel0[0, 0] = 1.0               # out row 0 <- in row 0 (top mirror)
    # t=0 top-mirror sel0 passes share their rhs slice with the identity
    # passes, so the sel0 terms fold into merged bands (3 fewer matmuls per
    # chain at t=0); b_iK0 doubles for both den and A chains (b_aiK == b_iK).
    # Seam bands: row seams across 128-row tiles use a 3-partition matmul
    # over a gathered seam tile (partition 0/1/2 = prev tile's row-127
    # gS[J1]/gSE[J0]/gSW[J2] slices) instead of three full sel matmuls.
    seamD = np.zeros((P, P), np.float32)
    seamD[0, 0] = WE
    seamD[1, 0] = WK
    seamD[2, 0] = WK
    d = {"b_iE": WE * I, "b_isE": WE * (I + S), "b_iK": WK * I,
         "b_sK": WK * S, "b_seamD": seamD, "b_seamA": -seamD,
         "b_isE0": WE * (I + S + sel0), "b_iK0": WK * (I + sel0),
         "b_aiE": WE * I, "b_aniE": -WE * I, "b_ainsE": WE * (I - S),
         "b_aiK": WK * I, "b_ansK": -WK * S,
         "b_ainsE0": WE * (I - S + sel0)}
    return np.stack([d[k] for k in BAND_NAMES], axis=1)  # [P, 14, P]


# --- custom DVE op: t8 = A * recip1(den + ws0), A = in0 (PSUM), den = in1 --
_RECIP_OP = None


def _get_recip_op():
    """out = in0 * y1 with y1 = one-Newton-step reciprocal of (in1 + s0).

    in0 rides the PSUM read port (the A accumulator - reading it releases
    the A bank directly), in1 is the ACT-evacuated fp16 den in SBUF.
    """
    global _RECIP_OP
    if _RECIP_OP is not None:
        return _RECIP_OP
    from concourse import dve_ops as dvo
    from concourse.dve_spec import Spec, Src0, Src1, C0, C1, C2, AluOp, Bin, lower
    from concourse.dve_uop import DveOpSpec

    name = "RECIP1MULSW_WS0_ANT"
    xs = Src1 + C0
    nx = Bin(AluOp.BITWISE_NOT, xs, xs)
    y0 = nx * C1
    y1 = y0 * (C2 - xs * y0)
    body = y1 * Src0

    def _ref(in0, in1, c0, c1, c2):
        xs = np.ascontiguousarray(in1.astype(np.float32) + np.float32(c0))
        nx = (~xs.view(np.int32)).view(np.float32)
        y0 = nx * np.float32(c1)
        y1 = y0 * (np.float32(c2) - xs * y0)
        return (y1 * in0.astype(np.float32)).astype(np.float32)

    spec = Spec(body=body, reference=_ref)
    shas = {}
    for ver in ("v3", "v4"):
        try:
            s = DveOpSpec(name=name, opcode=None, uops=lower(spec, ver=ver),
                          rd1_en=True)
            shas[ver] = s.sha(ver)
        except Exception:
            pass
    op = dvo.DveOp(name, spec, subdim=False, uops_sha=shas)
    if name not in dvo._SUB_OPCODE_FOR_NAME:
        dvo.OPS.append(op)
        dvo._SUB_OPCODE_FOR_NAME[name] = dvo._CUSTOM_DVE_ROW_BASE + len(dvo.OPS) - 1
        dvo.CUSTOM_DVE_SPECS[name] = spec
        assert dvo._SUB_OPCODE_FOR_NAME[name] < 0x20
    _RECIP_OP = op
    return op


# Chebyshev-minimax seed constants (same as RECIPROCAL_APPROX_FAST)
_RC0 = -0.23549792
_RC1 = 2.0017324


def build():
    recip_op = _get_recip_op()
    nc = bacc.Bacc("TRN2", target_bir_lowering=False, debug=False)
    x_d = nc.dram_tensor("x", [H, C, W], F16, kind="ExternalInput")
    y_d = nc.dram_tensor("y", [H, C, W], F16, kind="ExternalOutput")

    bands_d = nc.inline_tensor(_bands_np().astype(MM_NP), "bands")

    xh = x_d.ap()   # [H, C, W], partition = image row
    yh = y_d.ap()

    with tile.TileContext(nc) as tc, ExitStack() as ctx:
        const = ctx.enter_context(tc.tile_pool(name="const", bufs=1))
        pp = ctx.enter_context(tc.tile_pool(name="pp", bufs=2))
        dp = ctx.enter_context(tc.tile_pool(name="dp", bufs=2))
        gp = ctx.enter_context(tc.tile_pool(name="gp", bufs=3))
        hp = ctx.enter_context(tc.tile_pool(name="hp", bufs=3))
        fin = ctx.enter_context(tc.tile_pool(name="fin", bufs=2))
        smp = ctx.enter_context(tc.tile_pool(name="smp", bufs=2))
        psp = ctx.enter_context(tc.tile_pool(name="psp", bufs=1, space="PSUM"))

        # --- constants (issued after tile 0's loads; see loop top) ---
        bands_t = const.tile([P, len(BAND_NAMES), P], MM_DT, tag="bands")
        B = {k: bands_t[:, i, :] for i, k in enumerate(BAND_NAMES)}
        I_SEAMD = BAND_NAMES.index("b_seamD")
        I_SEAMA = BAND_NAMES.index("b_seamA")

        tiles = {}

        def issue_loads(t):
            r0 = t * P
            pma = pp.tile([P, C, WP], F16, tag="pma", bufs=4, name=f"pma_{t}")
            nc.sync.dma_start(out=pma[:, :, IM0 : IM0 + W], in_=xh[r0 : r0 + P])
            pda = pp.tile([P, C, WP], F16, tag="pda", bufs=4, name=f"pda_{t}")
            # tile 0's pda rides the ACT HWDGE queue so both tile-0 loads run
            # in parallel; everything else stays on the SP queue.
            eng = nc.scalar if t == 0 else nc.sync
            if t < NT - 1:
                eng.dma_start(out=pda[:, :, IM0 : IM0 + W], in_=xh[r0 + 1 : r0 + P + 1])
            else:
                eng.dma_start(out=pda[: P - 1, :, IM0 : IM0 + W], in_=xh[r0 + 1 : H])
                # reflect: image row 512 -> row 510
                eng.dma_start(out=pda[P - 1 : P, :, IM0 : IM0 + W], in_=xh[H - 2 : H - 1])
            tiles[t] = (pma, pda)

        prev_sm = None     # (sm_d, sm_a) seam tiles gathered from tile t-1
        next_sm = None
        nxt_d = None       # tile t's d_all, pre-allocated (+ d_s hoisted) at t-1
        prev_evac = None   # (den_ps, a_ps, pmid, r0) of previous tile
        for t in range(NT + 1):
            if t == 0:
                # full-width ping-pong seam tiles: rows 0-2 carry the
                # gathered seam data, rows 3-127 are zeroed ONCE here on the
                # DVE (idle during the first DMA wave anyway) so the seam
                # matmuls keep a full 128-partition rhs (FWL stays enabled;
                # zero band rows kill the dead partitions).
                sm_all = [smp.tile([P, C, W], F16, tag=f"sm{i}", bufs=1,
                                   name=f"sm{i}") for i in range(4)]
                for smt in sm_all:
                    nc.vector.memset(smt, 0.0)
                # dummy matmuls on an UNINITIALIZED garbage tile: no DMA
                # dependency, so the PE HAM clock-gate warm-up (needs ~3.4us
                # of activity) starts at kernel t~0 instead of after the
                # bands load.  Short 128-col passes so the queue drains just
                # as the first real chain's inputs land.
                garb = const.tile([P, P], F16, tag="garb")
                nc.gpsimd.memset(garb, 0.0)
                ps_scr = psp.tile([P, W], F32, tag="den0", bufs=2, name="ps_scr")
                for _ in range(80):
                    nc.tensor.matmul(ps_scr[:, :P], garb, garb, start=True, stop=True)
                issue_loads(0)
                # preload the erf_derivative ACT table while pma0 streams in
                # (reads its own uninitialized tile: value is discarded, only
                # the PSEUDO_LOAD_ACT_FUNC_SET side effect matters)
                act_scr = const.tile([P, 2], F16, tag="act_scr")
                nc.scalar.activation(act_scr, act_scr,
                                     AF.Derivative_Erf, bias=0.0, scale=SCALE_SQ)
                # bands AFTER the tile-0 loads on sync: the first DMA wave is
                # only pma0 (SP) + pda0 (ACT) so tile-0 subs start earliest;
                # bands land just before the first real chain matmul
                nc.sync.dma_start(out=bands_t, in_=bands_d.ap())
                issue_loads(1)
                issue_loads(2)
                issue_loads(3)

            if t < NT:
                r0 = t * P
                pma, pda = tiles.pop(t)
                cen = pma[:, :, J1 : J1 + W]

                # --- D_k = P(x+k) - P(x), fp16, all 4-byte aligned -> DVE 2x.
                # Offsets live in ONE parent tile [P, 4(k), C, WP] (slot
                # order S,E,SE,SW).
                hoisted = nxt_d is not None
                d_all = nxt_d if hoisted else dp.tile([P, 4, C, WP], F16,
                                                      tag="d_all", name=f"d_all_{t}")
                dS, dE, dSE, dSW = (d_all[:, k] for k in range(4))
                if t == 0:
                    nc.gpsimd.memset(d_all[:, :, :, 0:J1], 0.0)
                    nc.gpsimd.memset(d_all[:, :, :, J1 + W : WP], 0.0)
                # tiny pad ops: on gpsimd at steady state (keeps DVE free),
                # on DVE for tile 0 where gpsimd latency sits on the serial
                # D->G ramp (DVE does them in ~140ns)
                pe_ = nc.vector if t == 0 else nc.gpsimd
                if not hoisted:
                    nc.vector.tensor_sub(dS[:, :, J1 : J1 + W], pda[:, :, IM0 : IM0 + W], cen)
                pmb = pp.tile([P, C, WP], F16, tag="pmb", bufs=2, name=f"pmb_{t}")
                pdb = pp.tile([P, C, WP], F16, tag="pdb", bufs=2, name=f"pdb_{t}")
                # pmb copy + its pad right before d_e so gE is produced as
                # early as possible on tile 0 (shortens the ramp)
                nc.vector.tensor_copy(pmb[:, :, IMB : IMB + W], pma[:, :, IM0 : IM0 + W])
                pe_.tensor_copy(pmb[:, :, IMB + W : IMB + W + 1],
                                pmb[:, :, IMB + W - 2 : IMB + W - 1])
                nc.vector.tensor_sub(dE[:, :, J1 : J1 + W], pmb[:, :, IMB + 1 : IMB + 1 + W], cen)
                # D_E(h,-1) = -D_E(h,0) right after d_e: gE only waits this
                pe_.tensor_scalar_mul(dE[:, :, J0 : J0 + 1],
                                      dE[:, :, J1 : J1 + 1], -1.0)
                nc.vector.tensor_copy(pdb[:, :, IMB : IMB + W], pda[:, :, IM0 : IM0 + W])
                pe_.tensor_copy(pdb[:, :, IMB - 1 : IMB],
                                pdb[:, :, IMB + 1 : IMB + 2])
                pe_.tensor_copy(pdb[:, :, IMB + W : IMB + W + 1],
                                pdb[:, :, IMB + W - 2 : IMB + W - 1])
                nc.vector.tensor_sub(dSE[:, :, J1 : J1 + W], pdb[:, :, IMB + 1 : IMB + 1 + W], cen)
                nc.vector.tensor_sub(dSW[:, :, J1 : J1 + W], pdb[:, :, IMB - 1 : IMB - 1 + W], cen)
                # cross mirrors: D_SE(h,-1) = D_SW(h,1); D_SW(h,W) = D_SE(h,W-2)
                pe_.tensor_copy(dSE[:, :, J0 : J0 + 1], dSW[:, :, J2 : J2 + 1])
                pe_.tensor_copy(dSW[:, :, J2 + W - 1 : J2 + W],
                                dSE[:, :, J2 + W - 3 : J2 + W - 2])

            if t < NT:
                # --- kappa*exp(-D^2/(2s^2)): one ACT pass per offset so each
                # G lands as soon as its D is ready ---
                g_all = gp.tile([P, 4, C, WP], MM_DT, tag="g_all", name=f"g_all_{t}")
                for k in range(4):
                    nc.scalar.activation(g_all[:, k], d_all[:, k],
                                         AF.Derivative_Erf, bias=0.0, scale=SCALE_SQ)
                gS, gE, gSE, gSW = (g_all[:, k] for k in range(4))

            if t >= 1:
                # --- evac of previous tile: y = c + A * recip1(den + ws0) ---
                # den -> SBUF on ACT (sits AFTER DerivErf(t) in the scalar
                # FIFO so its wait on PE(t-1) never delays the G chain; den
                # chains finish early so this is never the critical read).
                # The recip op then reads A straight from PSUM (in0),
                # releasing the A banks on the DVE itself.
                pden, pa, ppm, pr0 = prev_evac
                den_sb = fin.tile([P, C, W], F16, tag="den_sb", name=f"den_sb_{t-1}")
                for c in range(C):
                    nc.scalar.copy(den_sb[:, c, :], pden[c])

            if t >= 1:
                # channel-0 evac first: releases the a0 PSUM bank before
                # PE(t) reaches its A chains
                t8 = fin.tile([P, C, W], F16, tag="t8", name=f"t8_{t-1}")
                nc.vector._custom_dve(
                    recip_op, out=t8[:, 0, :], in0=pa[0], in1=den_sb[:, 0, :],
                    s0=WS0, s1=_RC0, imm2=_RC1)

            if t < NT:
                # --- H~_k = D_k * E_k (DVE TT, fp16 2x; ws folded in bands) ---
                h_all = hp.tile([P, 4, C, WP], MM_DT, tag="h_all", name=f"h_all_{t}")
                for k in range(4):
                    nc.vector.tensor_mul(h_all[:, k], d_all[:, k], g_all[:, k])
                hS, hE, hSE, hSW = (h_all[:, k] for k in range(4))

            if t < NT - 1:
                # hoist tile t+1's S-offset sub ahead of this iteration's
                # remaining evac ops in the DVE FIFO: gS(t+1) is what gates
                # the next tile's first den matmuls at the boundary
                pma_n, pda_n = tiles[t + 1]
                nxt_d = dp.tile([P, 4, C, WP], F16, tag="d_all",
                                name=f"d_all_{t+1}")
                if t + 1 == 1:
                    nc.gpsimd.memset(nxt_d[:, :, :, 0:J1], 0.0)
                    nc.gpsimd.memset(nxt_d[:, :, :, J1 + W : WP], 0.0)
                nc.vector.tensor_sub(nxt_d[:, 0, :, J1 : J1 + W],
                                     pda_n[:, :, IM0 : IM0 + W],
                                     pma_n[:, :, J1 : J1 + W])
            else:
                nxt_d = None

            if t < NT - 1:
                # gather this tile's row-127 seam slices for tile t+1 via the
                # SWDGE queue (latency tolerant: consumed a full tile-period
                # later).  One full-width matmul then replaces three
                # full-width sel matmuls per chain.
                sm_d, sm_a = sm_all[2 * (t % 2)], sm_all[2 * (t % 2) + 1]
                nc.gpsimd.dma_start(out=sm_d[0:1], in_=gS[P - 1 : P, :, J1 : J1 + W])
                nc.gpsimd.dma_start(out=sm_d[1:2], in_=gSE[P - 1 : P, :, J0 : J0 + W])
                nc.gpsimd.dma_start(out=sm_d[2:3], in_=gSW[P - 1 : P, :, J2 : J2 + W])
                nc.gpsimd.dma_start(out=sm_a[0:1], in_=hS[P - 1 : P, :, J1 : J1 + W])
                nc.gpsimd.dma_start(out=sm_a[1:2], in_=hSE[P - 1 : P, :, J0 : J0 + W])
                nc.gpsimd.dma_start(out=sm_a[2:3], in_=hSW[P - 1 : P, :, J2 : J2 + W])
                next_sm = (sm_d, sm_a)

            if t >= 1:
                # channel-1 evac right after the H muls: frees the a1 PSUM
                # bank before PE(t) reaches its second A chain
                nc.vector._custom_dve(
                    recip_op, out=t8[:, 1, :], in0=pa[1], in1=den_sb[:, 1, :],
                    s0=WS0, s1=_RC0, imm2=_RC1)

            if t >= 1:
                # rest of the evac after H(t) in the vector FIFO
                yt = fin.tile([P, C, W], F16, tag="yt", name=f"yt_{t-1}")
                if t - 1 == NT - 1:
                    # last tile: per-channel finish + split stores so the
                    # final (smallest) store starts as early as possible
                    nc.vector.tensor_add(yt[:, 0, :], t8[:, 0, :],
                                         ppm[:, 0, J1 : J1 + W])
                    nc.sync.dma_start(out=yh[pr0 : pr0 + P, 0], in_=yt[:, 0, :])
                    nc.vector.tensor_add(yt[:, 1, :], t8[:, 1, :],
                                         ppm[:, 1, J1 : J1 + W])
                    nc.sync.dma_start(out=yh[pr0 : pr0 + P, 1], in_=yt[:, 1, :])
                    nc.vector._custom_dve(
                        recip_op, out=t8[:, 2, :], in0=pa[2], in1=den_sb[:, 2, :],
                        s0=WS0, s1=_RC0, imm2=_RC1)
                    nc.vector.tensor_add(yt[:, 2, :], t8[:, 2, :],
                                         ppm[:, 2, J1 : J1 + W])
                    nc.sync.dma_start(out=yh[pr0 : pr0 + P, 2], in_=yt[:, 2, :])
                else:
                    nc.vector._custom_dve(
                        recip_op, out=t8[:, 2, :], in0=pa[2], in1=den_sb[:, 2, :],
                        s0=WS0, s1=_RC0, imm2=_RC1)
                    nc.vector.tensor_add(yt, t8, ppm[:, :, J1 : J1 + W])
                    nc.sync.dma_start(out=yh[pr0 : pr0 + P], in_=yt)

            if t < NT:
                # --- PSUM accumulation chains (PE, fp16) ---
                den_ps = [psp.tile([P, W], F32, tag=f"den{c}", name=f"den{c}_{t}",
                                    bufs=2 if c <= 1 else 1) for c in range(C)]
                a_ps = [psp.tile([P, W], F32, tag=f"a{c}", name=f"a{c}_{t}")
                        for c in range(C)]

                def sl(ap, c, j):
                    return ap[:, c, j : j + W]

                def den_chain(c):
                    dn = den_ps[c]
                    # den chain (ws0 folded into the evac custom op; spatial
                    # weights folded into the bands; t=0 top-mirror sel0
                    # terms folded into the b_*0 bands)
                    nc.tensor.matmul(dn, B["b_isE0" if t == 0 else "b_isE"],
                                     sl(gS, c, J1), start=True, stop=False)
                    nc.tensor.matmul(dn, B["b_iE"], sl(gE, c, J1), start=False, stop=False)
                    nc.tensor.matmul(dn, B["b_iE"], sl(gE, c, J0), start=False, stop=False)
                    if t >= 1:
                        # row seam: one matmul over the gathered prev-tile
                        # row-127 slices (always ready early)
                        nc.tensor.matmul(dn, B["b_seamD"],
                                         prev_sm[0][:, c, :], start=False, stop=False)
                    bik = B["b_iK0" if t == 0 else "b_iK"]
                    nc.tensor.matmul(dn, bik, sl(gSE, c, J1), start=False, stop=False)
                    nc.tensor.matmul(dn, B["b_sK"], sl(gSE, c, J0), start=False, stop=False)
                    nc.tensor.matmul(dn, bik, sl(gSW, c, J1), start=False, stop=False)
                    nc.tensor.matmul(dn, B["b_sK"], sl(gSW, c, J2), start=False, stop=True)

                def a_chain(c):
                    # A chain (spatial weights folded into the bands)
                    an = a_ps[c]
                    nc.tensor.matmul(an, B["b_ainsE0" if t == 0 else "b_ainsE"],
                                     sl(hS, c, J1), start=True, stop=False)
                    nc.tensor.matmul(an, B["b_aiE"], sl(hE, c, J1), start=False, stop=False)
                    nc.tensor.matmul(an, B["b_aniE"], sl(hE, c, J0), start=False, stop=False)
                    if t >= 1:
                        nc.tensor.matmul(an, B["b_seamA"],
                                         prev_sm[1][:, c, :], start=False, stop=False)
                    baik = B["b_iK0" if t == 0 else "b_aiK"]
                    nc.tensor.matmul(an, baik, sl(hSE, c, J1), start=False, stop=False)
                    nc.tensor.matmul(an, B["b_ansK"], sl(hSE, c, J0), start=False, stop=False)
                    nc.tensor.matmul(an, baik, sl(hSW, c, J1), start=False, stop=False)
                    nc.tensor.matmul(an, B["b_ansK"], sl(hSW, c, J2), start=False, stop=True)

                if t == 0:
                    # tile 0: the G/H tiles trickle out of the serial
                    # D->G->H ramp, so emit term-major (3 channels per term)
                    den_terms = [(B["b_isE0"], gS, J1), (B["b_iE"], gE, J1),
                                 (B["b_iE"], gE, J0), (B["b_iK0"], gSE, J1),
                                 (B["b_sK"], gSE, J0), (B["b_iK0"], gSW, J1),
                                 (B["b_sK"], gSW, J2)]
                    a_terms = [(B["b_ainsE0"], hS, J1), (B["b_aiE"], hE, J1),
                               (B["b_aniE"], hE, J0), (B["b_iK0"], hSE, J1),
                               (B["b_ansK"], hSE, J0), (B["b_iK0"], hSW, J1),
                               (B["b_ansK"], hSW, J2)]
                    for terms, ps in ((den_terms, den_ps), (a_terms, a_ps)):
                        nterm = len(terms)
                        for k, (band, srct, j) in enumerate(terms):
                            for c in range(C):
                                nc.tensor.matmul(ps[c], band, sl(srct, c, j),
                                                 start=(k == 0),
                                                 stop=(k == nterm - 1))
                elif t == NT - 1:
                    # last tile: interleave den/A per channel so each
                    # channel's evac (recip, add, store) can start while the
                    # PE still works on later channels - cuts the tail
                    for c in range(C):
                        den_chain(c)
                        a_chain(c)
                else:
                    # den chains before A chains: single-buffered A banks are
                    # released by the evac just in time
                    for c in range(C):
                        den_chain(c)
                    for c in range(C):
                        a_chain(c)

                prev_sm = next_sm
                prev_evac = (den_ps, a_ps, pma, r0)

    nc.compile()
    return nc


_NC_CACHE = None


def _get_nc():
    global _NC_CACHE
    if _NC_CACHE is None:
        _NC_CACHE = build()
    return _NC_CACHE


def kernel(batch_img: np.ndarray) -> np.ndarray:
    assert batch_img.shape == (8, C, H, W), batch_img.shape
    # host-side prep: fp16 + [H, C, W] layout per image
    x = np.ascontiguousarray(
        np.asarray(batch_img, dtype=np.float16).transpose(0, 2, 1, 3))
    nc = _get_nc()
    in_maps = [{"x": x[b]} for b in range(8)]
    r = run_bass_kernel_spmd(nc, in_maps, core_ids=list(range(8)))
    out = np.stack([r.results[b]["y"] for b in range(8)], axis=0)  # [8,H,C,W]
    return np.ascontiguousarray(out.transpose(0, 2, 1, 3)).astype(np.float32)


if __name__ == "__main__":
    rng = np.random.default_rng(0)
    img = rng.random((8, C, H, W), np.float32)
    y = kernel(img)
    print("ran ok", y.shape, y.dtype)



# revision 62
# speedup vs baseline: 1.0035x; 1.0035x over previous
"""Bilateral filter 3x3 (sigma_space = sigma_color = 0.8) on 8 TRN2 NeuronCores.

Sharding: pure data parallelism - one batch image [3, 512, 512] per core.

Math (per core), with the color-normalization cancelled:
  out = c + A/den
    den(x) = ws0 + sum_{k in HP} [G_k(x) + G_k(x-k)]
    A(x)   =       sum_{k in HP} [H_k(x) - H_k(x-k)]
  where HP = {E=(0,1), S=(1,0), SE=(1,1), SW=(1,-1)},
    D_k = p~(x+k) - p(x),  G_k = ws_k * exp(-D_k^2 / (2 s^2)),  H_k = D_k * G_k.

Perf design (~107.9us f32 baseline -> ~80us profiled here):
  - fp16 on-chip: host converts + transposes input to [H,C,W] fp16; output is
    fp16, upcast on the host.  DVE tensor_tensor runs in 2x_1P packed mode,
    HBM traffic halves, PE streams fp16 rhs at 1 col/cycle.
  - G in ONE ACT pass per offset via Derivative_Erf(D*s) =
    (2/sqrt(pi))*exp(-D^2/(2 sigma^2)); kappa and the per-offset spatial
    weights fold into the PE band matrices of BOTH chains (no Square pass,
    no exp bias, no extra muls).  The erf_derivative table is preloaded
    during the DMA ramp by a throwaway activation.
  - rows are loaded at both column parities (A tiles image@col2, B tiles
    image@col3, derived on DVE with alignment-free 2x_2P copies) so all four
    subs are 4-byte aligned and hit 2x.  GPSIMD handles only tiny pad ops:
    bulk GPSIMD tensor ops run ~3.5ns/elem and their SBUF traffic slows
    concurrent DVE ops ~4x.
  - shifted terms accumulate on the TensorEngine with shift-band matmuls
    into PSUM (20 passes per channel-tile); row seams across 128-row tiles
    use selector bands against the previous tile's G/H; image boundaries use
    reflect-mirror identities applied in the D domain (D odd, G even).
  - evacuation: A copied PSUM->SBUF on ACT, then one fused custom DVE op
    t8 = reciprocal1(den + ws0) * A (bitwise-NOT exponent-flip seed + one
    Newton step, 7 ALU stages, ~0.4% max err), then one packed fp16 add
    y = t8 + center.  No separate reciprocal pass, no ws0*ones matmul.
  - only 2 HBM loads per tile + 1 store (split per-channel on the last tile
    so the final, smallest store launches earliest), all HWDGE DMAs kept to
    ~16 total: more DMAs overflow the 8 round-robin HWDGE completion-sem
    lanes and serialize the whole pipeline (observed 13-33us trigger
    stalls; 20+ DMAs cost tens of us).
  - dependency-ordered engine FIFOs: the S offset (no B-copy, no pad mirror)
    leads every stage so ACT starts early; the a_sb PSUM evac sits AFTER
    DerivErf in the scalar FIFO; the custom-op evac sits AFTER the H muls in
    the vector FIFO; den chains run before A chains on PE so single-buffered
    A banks are released by the evac just in time.
  - 22 dummy matmuls during the ramp keep the PE HAM clock-gate warm
    (2.4 GHz); all 240 production matmuls then issue at ~220 ns.

Layout: partition = image rows (4 tiles x 128 rows), free = (channel, width).
WP=522 cols; A tiles: image at cols 2..513; B tiles: image at cols 3..514.
"""
import math
import numpy as np
from contextlib import ExitStack

import concourse.bacc as bacc
import concourse.tile as tile
from concourse import mybir
from concourse.bass_utils import run_bass_kernel_spmd

F32 = mybir.dt.float32
F16 = mybir.dt.float16
MM_DT = F16
MM_NP = np.float16
AF = mybir.ActivationFunctionType
ALU = mybir.AluOpType

C, H, W = 3, 512, 512
P = 128                      # partitions per row-tile
NT = H // P                  # 4 row-tiles
WP = 522                     # col-padded width
IM0 = 2                      # first image column, A-parity tiles
IMB = 3                      # first image column, B-parity tiles
J0 = IM0 - 1
J1 = IM0
J2 = IM0 + 1

SIG = 0.8
TWO_SIG2 = 2.0 * SIG * SIG   # 1.28
SCALE_SQ = 1.0 / math.sqrt(TWO_SIG2)
KAPPA = 2.0 / math.sqrt(math.pi)     # DerivErf(u) = KAPPA * exp(-u^2)
_w1 = math.exp(-1.0 / TWO_SIG2)
_norm = (1.0 + 2.0 * _w1) ** 2
WS0 = 1.0 / _norm            # center weight
WS_E = _w1 / _norm           # edge
WS_K = _w1 * _w1 / _norm     # corner
WE = WS_E / KAPPA            # band / STT scale for edge offsets
WK = WS_K / KAPPA            # band / STT scale for corner offsets

BAND_NAMES = ["b_iE", "b_isE", "b_iK", "b_sK", "b_seamD", "b_seamA",
              "b_isE0", "b_iK0",
              "b_aiE", "b_aniE", "b_ainsE", "b_aiK", "b_ansK",
              "b_ainsE0"]


def _bands_np():
    I = np.eye(P, dtype=np.float32)
    S = np.zeros((P, P), np.float32)
    for m in range(1, P):
        S[m - 1, m] = 1.0          # lhsT[p, m]: out row m <- in row m-1
    sel = np.zeros((P, P), np.float32)
    sel[P - 1, 0] = 1.0            # out row 0 <- in row 127 (prev tile)
    sel0 = np.zeros((P, P), np.float32)
    sel0[0, 0] = 1.0               # out row 0 <- in row 0 (top mirror)
    # t=0 top-mirror sel0 passes share their rhs slice with the identity
    # passes, so the sel0 terms fold into merged bands (3 fewer matmuls per
    # chain at t=0); b_iK0 doubles for both den and A chains (b_aiK == b_iK).
    # Seam bands: row seams across 128-row tiles use a 3-partition matmul
    # over a gathered seam tile (partition 0/1/2 = prev tile's row-127
    # gS[J1]/gSE[J0]/gSW[J2] slices) instead of three full sel matmuls.
    seamD = np.zeros((P, P), np.float32)
    seamD[0, 0] = WE
    seamD[1, 0] = WK
    seamD[2, 0] = WK
    d = {"b_iE": WE * I, "b_isE": WE * (I + S), "b_iK": WK * I,
         "b_sK": WK * S, "b_seamD": seamD, "b_seamA": -seamD,
         "b_isE0": WE * (I + S + sel0), "b_iK0": WK * (I + sel0),
         "b_aiE": WE * I, "b_aniE": -WE * I, "b_ainsE": WE * (I - S),
         "b_aiK": WK * I, "b_ansK": -WK * S,
         "b_ainsE0": WE * (I - S + sel0)}
    return np.stack([d[k] for k in BAND_NAMES], axis=1)  # [P, 14, P]


# --- custom DVE op: t8 = A * recip1(den + ws0), A = in0 (PSUM), den = in1 --
_RECIP_OP = None


def _get_recip_op():
    """out = in0 * y1 with y1 = one-Newton-step reciprocal of (in1 + s0).

    in0 rides the PSUM read port (the A accumulator - reading it releases
    the A bank directly), in1 is the ACT-evacuated fp16 den in SBUF.
    """
    global _RECIP_OP
    if _RECIP_OP is not None:
        return _RECIP_OP
    from concourse import dve_ops as dvo
    from concourse.dve_spec import Spec, Src0, Src1, C0, C1, C2, AluOp, Bin, lower
    from concourse.dve_uop import DveOpSpec

    name = "RECIP1MULSW_WS0_ANT"
    xs = Src1 + C0
    nx = Bin(AluOp.BITWISE_NOT, xs, xs)
    y0 = nx * C1
    y1 = y0 * (C2 - xs * y0)
    body = y1 * Src0

    def _ref(in0, in1, c0, c1, c2):
        xs = np.ascontiguousarray(in1.astype(np.float32) + np.float32(c0))
        nx = (~xs.view(np.int32)).view(np.float32)
        y0 = nx * np.float32(c1)
        y1 = y0 * (np.float32(c2) - xs * y0)
        return (y1 * in0.astype(np.float32)).astype(np.float32)

    spec = Spec(body=body, reference=_ref)
    shas = {}
    for ver in ("v3", "v4"):
        try:
            s = DveOpSpec(name=name, opcode=None, uops=lower(spec, ver=ver),
                          rd1_en=True)
            shas[ver] = s.sha(ver)
        except Exception:
            pass
    op = dvo.DveOp(name, spec, subdim=False, uops_sha=shas)
    if name not in dvo._SUB_OPCODE_FOR_NAME:
        dvo.OPS.append(op)
        dvo._SUB_OPCODE_FOR_NAME[name] = dvo._CUSTOM_DVE_ROW_BASE + len(dvo.OPS) - 1
        dvo.CUSTOM_DVE_SPECS[name] = spec
        assert dvo._SUB_OPCODE_FOR_NAME[name] < 0x20
    _RECIP_OP = op
    return op


# Chebyshev-minimax seed constants (same as RECIPROCAL_APPROX_FAST)
_RC0 = -0.23549792
_RC1 = 2.0017324


def build():
    recip_op = _get_recip_op()
    nc = bacc.Bacc("TRN2", target_bir_lowering=False, debug=False)
    x_d = nc.dram_tensor("x", [H, C, W], F16, kind="ExternalInput")
    y_d = nc.dram_tensor("y", [H, C, W], F16, kind="ExternalOutput")

    bands_d = nc.inline_tensor(_bands_np().astype(MM_NP), "bands")

    xh = x_d.ap()   # [H, C, W], partition = image row
    yh = y_d.ap()

    with tile.TileContext(nc) as tc, ExitStack() as ctx:
        const = ctx.enter_context(tc.tile_pool(name="const", bufs=1))
        pp = ctx.enter_context(tc.tile_pool(name="pp", bufs=2))
        dp = ctx.enter_context(tc.tile_pool(name="dp", bufs=2))
        gp = ctx.enter_context(tc.tile_pool(name="gp", bufs=3))
        hp = ctx.enter_context(tc.tile_pool(name="hp", bufs=3))
        fin = ctx.enter_context(tc.tile_pool(name="fin", bufs=2))
        smp = ctx.enter_context(tc.tile_pool(name="smp", bufs=2))
        psp = ctx.enter_context(tc.tile_pool(name="psp", bufs=1, space="PSUM"))

        # --- constants (issued after tile 0's loads; see loop top) ---
        bands_t = const.tile([P, len(BAND_NAMES), P], MM_DT, tag="bands")
        B = {k: bands_t[:, i, :] for i, k in enumerate(BAND_NAMES)}
        I_SEAMD = BAND_NAMES.index("b_seamD")
        I_SEAMA = BAND_NAMES.index("b_seamA")

        tiles = {}

        def issue_loads(t):
            r0 = t * P
            pma = pp.tile([P, C, WP], F16, tag="pma", bufs=4, name=f"pma_{t}")
            nc.sync.dma_start(out=pma[:, :, IM0 : IM0 + W], in_=xh[r0 : r0 + P])
            pda = pp.tile([P, C, WP], F16, tag="pda", bufs=4, name=f"pda_{t}")
            # tile 0's pda rides the ACT HWDGE queue so both tile-0 loads run
            # in parallel; everything else stays on the SP queue.
            eng = nc.scalar if t == 0 else nc.sync
            if t < NT - 1:
                eng.dma_start(out=pda[:, :, IM0 : IM0 + W], in_=xh[r0 + 1 : r0 + P + 1])
            else:
                eng.dma_start(out=pda[: P - 1, :, IM0 : IM0 + W], in_=xh[r0 + 1 : H])
                # reflect: image row 512 -> row 510
                eng.dma_start(out=pda[P - 1 : P, :, IM0 : IM0 + W], in_=xh[H - 2 : H - 1])
            tiles[t] = (pma, pda)

        def emit_d_prod(n, da, pman, pdan, skip_s):
            """full D production for tile n: subs + parity copies + pads.
            Tiny pad ops on DVE for tile 0 (gpsimd latency would sit on the
            serial ramp), gpsimd otherwise."""
            cenn = pman[:, :, J1 : J1 + W]
            dSn, dEn, dSEn, dSWn = (da[:, k] for k in range(4))
            pe_ = nc.vector if n == 0 else nc.gpsimd
            if not skip_s:
                nc.vector.tensor_sub(dSn[:, :, J1 : J1 + W],
                                     pdan[:, :, IM0 : IM0 + W], cenn)
            pmb = pp.tile([P, C, WP], F16, tag="pmb", bufs=2, name=f"pmb_{n}")
            pdb = pp.tile([P, C, WP], F16, tag="pdb", bufs=2, name=f"pdb_{n}")
            nc.vector.tensor_copy(pmb[:, :, IMB : IMB + W], pman[:, :, IM0 : IM0 + W])
            pe_.tensor_copy(pmb[:, :, IMB + W : IMB + W + 1],
                            pmb[:, :, IMB + W - 2 : IMB + W - 1])
            nc.vector.tensor_sub(dEn[:, :, J1 : J1 + W],
                                 pmb[:, :, IMB + 1 : IMB + 1 + W], cenn)
            pe_.tensor_scalar_mul(dEn[:, :, J0 : J0 + 1],
                                  dEn[:, :, J1 : J1 + 1], -1.0)
            nc.vector.tensor_copy(pdb[:, :, IMB : IMB + W], pdan[:, :, IM0 : IM0 + W])
            pe_.tensor_copy(pdb[:, :, IMB - 1 : IMB], pdb[:, :, IMB + 1 : IMB + 2])
            pe_.tensor_copy(pdb[:, :, IMB + W : IMB + W + 1],
                            pdb[:, :, IMB + W - 2 : IMB + W - 1])
            nc.vector.tensor_sub(dSEn[:, :, J1 : J1 + W],
                                 pdb[:, :, IMB + 1 : IMB + 1 + W], cenn)
            nc.vector.tensor_sub(dSWn[:, :, J1 : J1 + W],
                                 pdb[:, :, IMB - 1 : IMB - 1 + W], cenn)
            pe_.tensor_copy(dSEn[:, :, J0 : J0 + 1], dSWn[:, :, J2 : J2 + 1])
            pe_.tensor_copy(dSWn[:, :, J2 + W - 1 : J2 + W],
                            dSEn[:, :, J2 + W - 3 : J2 + W - 2])

        prev_sm = None     # (sm_d, sm_a) seam tiles gathered from tile t-1
        next_sm = None
        nxt_d = None       # tile t's d_all, pre-allocated (+ d_s hoisted) at t-1
        prev_evac = None   # (den_ps, a_ps, pmid, r0) of previous tile
        for t in range(NT + 1):
            if t == 0:
                # full-width ping-pong seam tiles: rows 0-2 carry the
                # gathered seam data, rows 3-127 are zeroed ONCE here on the
                # DVE (idle during the first DMA wave anyway) so the seam
                # matmuls keep a full 128-partition rhs (FWL stays enabled;
                # zero band rows kill the dead partitions).
                sm_all = [smp.tile([P, C, W], F16, tag=f"sm{i}", bufs=1,
                                   name=f"sm{i}") for i in range(4)]
                for smt in sm_all:
                    nc.vector.memset(smt, 0.0)
                # dummy matmuls on an UNINITIALIZED garbage tile: no DMA
                # dependency, so the PE HAM clock-gate warm-up (needs ~3.4us
                # of activity) starts at kernel t~0 instead of after the
                # bands load.  Short 128-col passes so the queue drains just
                # as the first real chain's inputs land.
                garb = const.tile([P, P], F16, tag="garb")
                nc.gpsimd.memset(garb, 0.0)
                ps_scr = psp.tile([P, W], F32, tag="den0", bufs=2, name="ps_scr")
                for _ in range(80):
                    nc.tensor.matmul(ps_scr[:, :P], garb, garb, start=True, stop=True)
                issue_loads(0)
                # preload the erf_derivative ACT table while pma0 streams in
                # (reads its own uninitialized tile: value is discarded, only
                # the PSEUDO_LOAD_ACT_FUNC_SET side effect matters)
                act_scr = const.tile([P, 2], F16, tag="act_scr")
                nc.scalar.activation(act_scr, act_scr,
                                     AF.Derivative_Erf, bias=0.0, scale=SCALE_SQ)
                # bands AFTER the tile-0 loads on sync: the first DMA wave is
                # only pma0 (SP) + pda0 (ACT) so tile-0 subs start earliest;
                # bands land just before the first real chain matmul
                nc.sync.dma_start(out=bands_t, in_=bands_d.ap())
                issue_loads(1)
                issue_loads(2)
                issue_loads(3)

            if t < NT:
                r0 = t * P
                pma, pda = tiles.pop(t)
                cen = pma[:, :, J1 : J1 + W]

                # --- D_k = P(x+k) - P(x), fp16, all 4-byte aligned -> DVE 2x.
                # Offsets live in ONE parent tile [P, 4(k), C, WP] (slot
                # order S,E,SE,SW).
                hoisted = nxt_d is not None
                d_all = nxt_d if hoisted else dp.tile([P, 4, C, WP], F16,
                                                      tag="d_all", name=f"d_all_{t}")
                dS, dE, dSE, dSW = (d_all[:, k] for k in range(4))
                if t == 0:
                    nc.gpsimd.memset(d_all[:, :, :, 0:J1], 0.0)
                    nc.gpsimd.memset(d_all[:, :, :, J1 + W : WP], 0.0)
                if not hoisted:
                    emit_d_prod(t, d_all, pma, pda, skip_s=False)

            if t < NT:
                # --- kappa*exp(-D^2/(2s^2)): one ACT pass per offset so each
                # G lands as soon as its D is ready ---
                g_all = gp.tile([P, 4, C, WP], MM_DT, tag="g_all", name=f"g_all_{t}")
                for k in range(4):
                    nc.scalar.activation(g_all[:, k], d_all[:, k],
                                         AF.Derivative_Erf, bias=0.0, scale=SCALE_SQ)
                gS, gE, gSE, gSW = (g_all[:, k] for k in range(4))

            if t >= 1:
                # --- evac of previous tile: y = c + A * recip1(den + ws0) ---
                # den -> SBUF on ACT (sits AFTER DerivErf(t) in the scalar
                # FIFO so its wait on PE(t-1) never delays the G chain; den
                # chains finish early so this is never the critical read).
                # The recip op then reads A straight from PSUM (in0),
                # releasing the A banks on the DVE itself.
                pden, pa, ppm, pr0 = prev_evac
                den_sb = fin.tile([P, C, W], F16, tag="den_sb", name=f"den_sb_{t-1}")
                for c in range(C):
                    nc.scalar.copy(den_sb[:, c, :], pden[c])

            if t >= 1:
                # channel-0 evac first: releases the a0 PSUM bank before
                # PE(t) reaches its A chains
                t8 = fin.tile([P, C, W], F16, tag="t8", name=f"t8_{t-1}")
                nc.vector._custom_dve(
                    recip_op, out=t8[:, 0, :], in0=pa[0], in1=den_sb[:, 0, :],
                    s0=WS0, s1=_RC0, imm2=_RC1)

            if t < NT:
                # --- H~_k = D_k * E_k (DVE TT, fp16 2x; ws folded in bands) ---
                h_all = hp.tile([P, 4, C, WP], MM_DT, tag="h_all", name=f"h_all_{t}")
                for k in range(4):
                    nc.vector.tensor_mul(h_all[:, k], d_all[:, k], g_all[:, k])
                hS, hE, hSE, hSW = (h_all[:, k] for k in range(4))

            if t < NT - 1:
                # hoist tile t+1's S-offset sub ahead of this iteration's
                # remaining evac ops in the DVE FIFO: gS(t+1) is what gates
                # the next tile's first den matmuls at the boundary
                pma_n, pda_n = tiles[t + 1]
                nxt_d = dp.tile([P, 4, C, WP], F16, tag="d_all",
                                name=f"d_all_{t+1}")
                if t + 1 == 1:
                    nc.gpsimd.memset(nxt_d[:, :, :, 0:J1], 0.0)
                    nc.gpsimd.memset(nxt_d[:, :, :, J1 + W : WP], 0.0)
                nc.vector.tensor_sub(nxt_d[:, 0, :, J1 : J1 + W],
                                     pda_n[:, :, IM0 : IM0 + W],
                                     pma_n[:, :, J1 : J1 + W])
                emit_d_prod(t + 1, nxt_d, pma_n, pda_n, skip_s=True)
            else:
                nxt_d = None

            if t < NT - 1:
                # gather this tile's row-127 seam slices for tile t+1 via the
                # SWDGE queue (latency tolerant: consumed a full tile-period
                # later).  One full-width matmul then replaces three
                # full-width sel matmuls per chain.
                sm_d, sm_a = sm_all[2 * (t % 2)], sm_all[2 * (t % 2) + 1]
                nc.gpsimd.dma_start(out=sm_d[0:1], in_=gS[P - 1 : P, :, J1 : J1 + W])
                nc.gpsimd.dma_start(out=sm_d[1:2], in_=gSE[P - 1 : P, :, J0 : J0 + W])
                nc.gpsimd.dma_start(out=sm_d[2:3], in_=gSW[P - 1 : P, :, J2 : J2 + W])
                nc.gpsimd.dma_start(out=sm_a[0:1], in_=hS[P - 1 : P, :, J1 : J1 + W])
                nc.gpsimd.dma_start(out=sm_a[1:2], in_=hSE[P - 1 : P, :, J0 : J0 + W])
                nc.gpsimd.dma_start(out=sm_a[2:3], in_=hSW[P - 1 : P, :, J2 : J2 + W])
                next_sm = (sm_d, sm_a)

            if t >= 1:
                # channel-1 evac right after the H muls: frees the a1 PSUM
                # bank before PE(t) reaches its second A chain
                nc.vector._custom_dve(
                    recip_op, out=t8[:, 1, :], in0=pa[1], in1=den_sb[:, 1, :],
                    s0=WS0, s1=_RC0, imm2=_RC1)

            if t >= 1:
                # rest of the evac after H(t) in the vector FIFO
                yt = fin.tile([P, C, W], F16, tag="yt", name=f"yt_{t-1}")
                if t - 1 == NT - 1:
                    # last tile: per-channel finish + split stores so the
                    # final (smallest) store starts as early as possible
                    nc.vector.tensor_add(yt[:, 0, :], t8[:, 0, :],
                                         ppm[:, 0, J1 : J1 + W])
                    nc.sync.dma_start(out=yh[pr0 : pr0 + P, 0], in_=yt[:, 0, :])
                    nc.vector.tensor_add(yt[:, 1, :], t8[:, 1, :],
                                         ppm[:, 1, J1 : J1 + W])
                    nc.sync.dma_start(out=yh[pr0 : pr0 + P, 1], in_=yt[:, 1, :])
                    nc.vector._custom_dve(
                        recip_op, out=t8[:, 2, :], in0=pa[2], in1=den_sb[:, 2, :],
                        s0=WS0, s1=_RC0, imm2=_RC1)
                    nc.vector.tensor_add(yt[:, 2, :], t8[:, 2, :],
                                         ppm[:, 2, J1 : J1 + W])
                    nc.sync.dma_start(out=yh[pr0 : pr0 + P, 2], in_=yt[:, 2, :])
                else:
                    nc.vector._custom_dve(
                        recip_op, out=t8[:, 2, :], in0=pa[2], in1=den_sb[:, 2, :],
                        s0=WS0, s1=_RC0, imm2=_RC1)
                    nc.vector.tensor_add(yt, t8, ppm[:, :, J1 : J1 + W])
                    nc.sync.dma_start(out=yh[pr0 : pr0 + P], in_=yt)

            if t < NT:
                # --- PSUM accumulation chains (PE, fp16) ---
                den_ps = [psp.tile([P, W], F32, tag=f"den{c}", name=f"den{c}_{t}",
                                    bufs=2 if c <= 1 else 1) for c in range(C)]
                a_ps = [psp.tile([P, W], F32, tag=f"a{c}", name=f"a{c}_{t}")
                        for c in range(C)]

                def sl(ap, c, j):
                    return ap[:, c, j : j + W]

                def den_chain(c):
                    dn = den_ps[c]
                    # den chain (ws0 folded into the evac custom op; spatial
                    # weights folded into the bands; t=0 top-mirror sel0
                    # terms folded into the b_*0 bands)
                    nc.tensor.matmul(dn, B["b_isE0" if t == 0 else "b_isE"],
                                     sl(gS, c, J1), start=True, stop=False)
                    nc.tensor.matmul(dn, B["b_iE"], sl(gE, c, J1), start=False, stop=False)
                    nc.tensor.matmul(dn, B["b_iE"], sl(gE, c, J0), start=False, stop=False)
                    if t >= 1:
                        # row seam: one matmul over the gathered prev-tile
                        # row-127 slices (always ready early)
                        nc.tensor.matmul(dn, B["b_seamD"],
                                         prev_sm[0][:, c, :], start=False, stop=False)
                    bik = B["b_iK0" if t == 0 else "b_iK"]
                    nc.tensor.matmul(dn, bik, sl(gSE, c, J1), start=False, stop=False)
                    nc.tensor.matmul(dn, B["b_sK"], sl(gSE, c, J0), start=False, stop=False)
                    nc.tensor.matmul(dn, bik, sl(gSW, c, J1), start=False, stop=False)
                    nc.tensor.matmul(dn, B["b_sK"], sl(gSW, c, J2), start=False, stop=True)

                def a_chain(c):
                    # A chain (spatial weights folded into the bands)
                    an = a_ps[c]
                    nc.tensor.matmul(an, B["b_ainsE0" if t == 0 else "b_ainsE"],
                                     sl(hS, c, J1), start=True, stop=False)
                    nc.tensor.matmul(an, B["b_aiE"], sl(hE, c, J1), start=False, stop=False)
                    nc.tensor.matmul(an, B["b_aniE"], sl(hE, c, J0), start=False, stop=False)
                    if t >= 1:
                        nc.tensor.matmul(an, B["b_seamA"],
                                         prev_sm[1][:, c, :], start=False, stop=False)
                    baik = B["b_iK0" if t == 0 else "b_aiK"]
                    nc.tensor.matmul(an, baik, sl(hSE, c, J1), start=False, stop=False)
                    nc.tensor.matmul(an, B["b_ansK"], sl(hSE, c, J0), start=False, stop=False)
                    nc.tensor.matmul(an, baik, sl(hSW, c, J1), start=False, stop=False)
                    nc.tensor.matmul(an, B["b_ansK"], sl(hSW, c, J2), start=False, stop=True)

                if t == 0:
                    # tile 0: the G/H tiles trickle out of the serial
                    # D->G->H ramp, so emit term-major (3 channels per term)
                    den_terms = [(B["b_isE0"], gS, J1), (B["b_iE"], gE, J1),
                                 (B["b_iE"], gE, J0), (B["b_iK0"], gSE, J1),
                                 (B["b_sK"], gSE, J0), (B["b_iK0"], gSW, J1),
                                 (B["b_sK"], gSW, J2)]
                    a_terms = [(B["b_ainsE0"], hS, J1), (B["b_aiE"], hE, J1),
                               (B["b_aniE"], hE, J0), (B["b_iK0"], hSE, J1),
                               (B["b_ansK"], hSE, J0), (B["b_iK0"], hSW, J1),
                               (B["b_ansK"], hSW, J2)]
                    for terms, ps in ((den_terms, den_ps), (a_terms, a_ps)):
                        nterm = len(terms)
                        for k, (band, srct, j) in enumerate(terms):
                            for c in range(C):
                                nc.tensor.matmul(ps[c], band, sl(srct, c, j),
                                                 start=(k == 0),
                                                 stop=(k == nterm - 1))
                elif t == NT - 1:
                    # last tile: interleave den/A per channel so each
                    # channel's evac (recip, add, store) can start while the
                    # PE still works on later channels - cuts the tail
                    for c in range(C):
                        den_chain(c)
                        a_chain(c)
                else:
                    # den chains before A chains: single-buffered A banks are
                    # released by the evac just in time
                    for c in range(C):
                        den_chain(c)
                    for c in range(C):
                        a_chain(c)

                prev_sm = next_sm
                prev_evac = (den_ps, a_ps, pma, r0)

    nc.compile()
    return nc


_NC_CACHE = None


def _get_nc():
    global _NC_CACHE
    if _NC_CACHE is None:
        _NC_CACHE = build()
    return _NC_CACHE


def kernel(batch_img: np.ndarray) -> np.ndarray:
    assert batch_img.shape == (8, C, H, W), batch_img.shape
    # host-side prep: fp16 + [H, C, W] layout per image
    x = np.ascontiguousarray(
        np.asarray(batch_img, dtype=np.float16).transpose(0, 2, 1, 3))
    nc = _get_nc()
    in_maps = [{"x": x[b]} for b in range(8)]
    r = run_bass_kernel_spmd(nc, in_maps, core_ids=list(range(8)))
    out = np.stack([r.results[b]["y"] for b in range(8)], axis=0)  # [8,H,C,W]
    return np.ascontiguousarray(out.transpose(0, 2, 1, 3)).astype(np.float32)


if __name__ == "__main__":
    rng = np.random.default_rng(0)
    img = rng.random((8, C, H, W), np.float32)
    y = kernel(img)
    print("ran ok", y.shape, y.dtype)

